# revision 1
# baseline (speedup 1.0000x reference)
"""NT-Xent (SimCLR contrastive) loss on Trainium2, sharded across 8 NeuronCores.

Each core computes a [512, 4096] row-slice of the similarity matrix
sim = zn_own^T . zn_all (fp8 DoubleRow matmuls, x16 fp8 scaling), with the
exp row-sums fused into ScalarE's activation accumulator, an exact fp8-level
diagonal recompute, and positives from a host-shipped partner slice. Host
sums the 8 scalar partials (the unshard step). No host arithmetic beyond
sharding/layout/dtype-cast of inputs and summing the per-core partials.

Schedule (the perf-critical part, ~60us vs the 72us it replaces):
  - inputs are host-pre-permuted to per-partition-contiguous [128, ...]
    layouts so each tensor loads with one large-descriptor DMA, all on the
    sync HWDGE queue (scalar-queue DMAs would stall ScalarE activations).
  - phase-hoisted emission: all per-block normalize work is emitted before
    any Gram matmul so the PE stream never stalls on a later block's fp8
    conversion; block 0's chain leads every engine queue.
  - block fp8 via bf16 normalize (2x DVE mode, one broadcast-rin
    tensor_tensor per block) + SWDGE cast-DMA, pipelined across blocks.
    The cast ROUNDS (it does not truncate): no pre-compensation, and the
    own-slice fp8 (DVE round-to-nearest) stays scale-symmetric with the
    blocks so the diagonal term cancels exactly.
  - finale pieces that only depend on the own slice (diag transpose, dexp,
    positives reduction) run during the main loop; the tail after the last
    exp is a short fused chain.
  - PE warmers bridge head idle (initial batch + a second batch after the
    own-slice ssq) so the HAM clock gate stays at 2.4 GHz through the ssq
    phase; work-pool rotation depth 3 gives the norm conveyor WAR slack.
  - one activation-table load: Ln/Exp pinned to natural_log_exp_and_others
    via a compile-time patch of bacc's table map.
"""

import numpy as np

B = 2048
D = 512
N2 = 2 * B              # 4096 total rows
NCORES = 8
RPC = N2 // NCORES      # 512 rows per core
KT = D // 128           # 4 contraction tiles
BLK = 1024              # column-block size
NBLK = N2 // BLK        # 4 blocks
TEMP = 0.1
SCALE = 1.0 / TEMP      # 10.0
FP8_SCALE = 16.0        # zn is stored as fp8(zn*16); sim256 = 256*sim
# SWDGE dtype-cast truncates toward zero; pre-scale by ~half an average
# e4m3 ULP so the truncated grid is centered. Applies to the block (cast-DMA)
# path only; the own slice converts on the DVE (round-to-nearest).
TRUNC_COMP = 1.0  # SWDGE cast rounds (does NOT truncate); no pre-compensation
LN_FP8_OWN = float(np.log(FP8_SCALE))
LN_FP8_BLK = float(np.log(FP8_SCALE * TRUNC_COMP))
NWARM = 28              # initial PE warmers during the first DMA wait

_CACHE = {}


def _patch_act_tables(nc, mybir):
    """Make Ln and Exp resolve to the shared natural_log_exp_and_others set
    so the compiler emits one ACT table load instead of thrashing."""
    from concourse import hw_specs

    tables = hw_specs.get_activation_tables(nc.m.arch)
    keep = "natural_log_exp_and_others"
    if keep not in tables:
        return
    F = mybir.ActivationFunctionType
    if F.Exp not in tables[keep] or F.Ln not in tables[keep]:
        return
    for name, fns in tables.items():
        if name != keep:
            fns.discard(F.Exp)
            fns.discard(F.Ln)


def _build():
    from concourse import bass, bacc, tile, mybir

    nc = bacc.Bacc("TRN2", target_bir_lowering=False, debug=False,
                   num_devices=NCORES)
    bf16 = mybir.dt.bfloat16
    f32 = mybir.dt.float32
    f8 = mybir.dt.float8e4
    F = mybir.ActivationFunctionType
    A = mybir.AluOpType
    AX = mybir.AxisListType
    DR = mybir.MatmulPerfMode.DoubleRow
    PSUM = bass.MemorySpace.PSUM

    # host-pre-permuted: per-partition-contiguous layouts for fast DMA
    zt = nc.dram_tensor("zt", [128, NBLK, KT, BLK], bf16,
                        kind="ExternalInput").ap()
    zown = nc.dram_tensor("zown", [128, KT, RPC], bf16,
                          kind="ExternalInput").ap()
    zpr = nc.dram_tensor("zpr", [128, KT, RPC], bf16,
                         kind="ExternalInput").ap()
    out = nc.dram_tensor("out", [1, 1], f32, kind="ExternalOutput").ap()

    with tile.TileContext(nc) as tc:
        with (
            tc.tile_pool(name="sb", bufs=1) as sb,
            tc.tile_pool(name="wrk", bufs=4) as wrk,
            tc.tile_pool(name="psA", bufs=1, space=PSUM) as psA,
            tc.tile_pool(name="psA1", bufs=1, space=PSUM) as psA1,
            tc.tile_pool(name="psB", bufs=2, space=PSUM) as psB,
        ):
            ones = sb.tile([128, 128], bf16, tag="ones")
            nc.vector.memset(ones[:], 1.0)
            bias_own = sb.tile([128, 1], f32, tag="bown")
            nc.vector.memset(bias_own[:], LN_FP8_OWN)
            bias_blk = sb.tile([128, 1], f32, tag="bblk")
            nc.vector.memset(bias_blk[:], LN_FP8_BLK)
            bias_10 = sb.tile([128, 1], f32, tag="b10")
            nc.vector.memset(bias_10[:], SCALE)

            # ---- input DMAs: one instruction each, all on the sync queue
            # (scalar-queue DMAs stall ScalarE activations behind them)
            zok = sb.tile([128, KT, RPC], bf16, tag="zok")
            zpk = sb.tile([128, KT, RPC], bf16, tag="zpk")
            zb = [sb.tile([128, KT, BLK], bf16, tag=f"zt{b}", name=f"zb{b}")
                  for b in range(NBLK)]
            nc.sync.dma_start(out=zb[0][:], in_=zt[:, 0])
            nc.sync.dma_start(out=zok[:], in_=zown)
            nc.sync.dma_start(out=zb[1][:], in_=zt[:, 1])
            nc.sync.dma_start(out=zpk[:], in_=zpr)
            nc.sync.dma_start(out=zb[2][:], in_=zt[:, 2])
            nc.sync.dma_start(out=zb[3][:], in_=zt[:, 3])

            warm = psA1.tile([128, RPC], f32, tag="pd")
            for _ in range(NWARM):
                nc.tensor.matmul(warm[:, 0:128], ones[:], ones[:],
                                 start=True, stop=True)

            zn16 = [sb.tile([128, KT, BLK], bf16, tag=f"zn16_{b}", name=f"zn16_{b}")
                    for b in range(NBLK)]
            zn8 = [sb.tile([128, KT, BLK], f8, tag=f"zn8_{b}", name=f"zn8_{b}")
                   for b in range(NBLK)]

            def norm_block(b):
                sq = wrk.tile([128, KT, BLK], bf16, tag="sq", name="sq")
                nc.vector.tensor_tensor(sq[:], zb[b][:], zb[b][:], A.mult)
                ps = psA.tile([128, BLK], f32, tag="ssq", name="ps")
                for k in range(KT):
                    for j in range(BLK // 512):
                        nc.tensor.matmul(ps[:, j * 512:(j + 1) * 512],
                                         ones[:], sq[:, k, j * 512:(j + 1) * 512],
                                         start=(k == 0), stop=(k == KT - 1))
                lns = wrk.tile([128, BLK], f32, tag="lns", name="lns")
                nc.scalar.activation(lns[:], ps[:], F.Ln)
                rin = wrk.tile([128, BLK], bf16, tag="rin", name="rin")
                nc.scalar.activation(rin[:], lns[:], F.Exp, scale=-0.5,
                                     bias=bias_blk[:])
                nc.vector.tensor_tensor(
                    zn16[b][:], zb[b][:],
                    rin[:].unsqueeze(1).broadcast_to([128, KT, BLK]), A.mult)
                nc.gpsimd.dma_start(out=zn8[b][:], in_=zn16[b][:])

            norm_block(0)

            # ---- own slice: norm -> fp8(zn*16) on DVE (needed early) ----
            zno = sb.tile([128, KT, RPC], f8, tag="zno")
            sqo = wrk.tile([128, KT, RPC], bf16, tag="sq_s")
            nc.vector.tensor_tensor(sqo[:], zok[:], zok[:], A.mult)
            pso = psA1.tile([128, RPC], f32, tag="pd")
            for k in range(KT):
                nc.tensor.matmul(pso[:], ones[:], sqo[:, k, :],
                                 start=(k == 0), stop=(k == KT - 1))
            warm2 = psA1.tile([128, RPC], f32, tag="pd")
            for _ in range(16):
                nc.tensor.matmul(warm2[:, 0:128], ones[:], ones[:],
                                 start=True, stop=True)
            lno = wrk.tile([128, RPC], f32, tag="lns_s")
            nc.scalar.activation(lno[:], pso[:], F.Ln)
            rino = wrk.tile([128, RPC], bf16, tag="rin_s")
            nc.scalar.activation(rino[:], lno[:], F.Exp, scale=-0.5,
                                 bias=bias_own[:])
            nc.vector.tensor_tensor(
                zno[:], zok[:],
                rino[:].unsqueeze(1).broadcast_to([128, KT, RPC]), A.mult)

            # ---- remaining block normalizes (hoisted) ----
            for b in range(1, NBLK):
                norm_block(b)

            # ---- partner norm (bf16 x1) + positives + diag: overlap with
            # the block phase; results only feed the cheap finale ----
            znp = sb.tile([128, KT, RPC], bf16, tag="znp")
            sqp = wrk.tile([128, KT, RPC], bf16, tag="sq_s")
            nc.vector.tensor_tensor(sqp[:], zpk[:], zpk[:], A.mult)
            psp = psA1.tile([128, RPC], f32, tag="pd")
            for k in range(KT):
                nc.tensor.matmul(psp[:], ones[:], sqp[:, k, :],
                                 start=(k == 0), stop=(k == KT - 1))
            lnp = wrk.tile([128, RPC], f32, tag="lns_s")
            nc.scalar.activation(lnp[:], psp[:], F.Ln)
            rinp = wrk.tile([128, RPC], bf16, tag="rin_s")
            nc.scalar.activation(rinp[:], lnp[:], F.Exp, scale=-0.5)
            nc.vector.tensor_tensor(
                znp[:], zpk[:],
                rinp[:].unsqueeze(1).broadcast_to([128, KT, RPC]), A.mult)

            pos_red = sb.tile([128, 1], f32, tag="posr")
            pp = psA1.tile([128, RPC], f32, tag="pd")
            for k in range(KT):
                pr = wrk.tile([128, RPC], bf16, tag="prod")
                nc.vector.tensor_tensor(pr[:], zno[:, k, :], znp[:, k, :],
                                        A.mult)
                nc.tensor.matmul(pp[:], ones[:], pr[:],
                                 start=(k == 0), stop=(k == KT - 1))
            nc.vector.tensor_reduce(pos_red[:], pp[:], AX.X, A.add)

            dg = psA1.tile([1, RPC], f32, tag="pd")
            for k in range(KT):
                pr = wrk.tile([128, RPC], bf16, tag="prod")
                nc.vector.tensor_tensor(pr[:], zno[:, k, :], zno[:, k, :],
                                        A.mult)
                nc.tensor.matmul(dg[:], ones[:, 0:1], pr[:],
                                 start=(k == 0), stop=(k == KT - 1))
            diag_row = sb.tile([1, RPC], bf16, tag="diagrow")
            nc.vector.tensor_scalar_add(diag_row[:], dg[:], -FP8_SCALE ** 2)

            # diag -> partition layout + dexp, done before the Gram tail
            dt = psA1.tile([128, RPC], f32, tag="pd")
            for m in range(4):
                nc.tensor.matmul(dt[:, m * 128:(m + 1) * 128],
                                 diag_row[0:1, m * 128:(m + 1) * 128],
                                 ones[0:1, :], start=True, stop=True)
            diag_part = sb.tile([128, 4], f32, tag="diagp")
            for m in range(4):
                nc.vector.tensor_copy(diag_part[:, m:m + 1],
                                      dt[:, m * 128:m * 128 + 1])
            dexp = sb.tile([128, 4], f32, tag="dexp")
            nc.scalar.activation(dexp[:], diag_part[:], F.Exp,
                                 scale=SCALE / (FP8_SCALE ** 2),
                                 bias=bias_10[:])

            # ---- Gram + fused exp row-sums ----
            rowp = sb.tile([128, 4, NBLK], f32, tag="rowp")
            for b in range(NBLK):
                for m in range(4):
                    pm = psB.tile([128, BLK], f32, tag="mm")
                    for g in range(KT // 2):
                        lhsT = zno[:, 2 * g:2 * g + 2, m * 128:(m + 1) * 128]
                        for j in range(BLK // 512):
                            nc.tensor.matmul(
                                pm[:, j * 512:(j + 1) * 512],
                                lhsT,
                                zn8[b][:, 2 * g:2 * g + 2, j * 512:(j + 1) * 512],
                                start=(g == 0), stop=(g == KT // 2 - 1),
                                perf_mode=DR)
                    scr = wrk.tile([128, BLK], bf16, tag="scr")
                    nc.scalar.activation(
                        scr[:], pm[:], F.Exp,
                        scale=SCALE / (FP8_SCALE ** 2),
                        accum_out=rowp[:, m, b:b + 1])

            # ---- finale: partial = sum_r ln(Z_r) - 10 * sum_r pos_r ----
            dexp4 = sb.tile([128, 4], f32, tag="dexp4")
            nc.vector.tensor_scalar_mul(dexp4[:], dexp[:], 1.0 / NBLK)
            zarg = sb.tile([128, 4], f32, tag="zarg")
            logz = sb.tile([128, 5], f32, tag="logz")
            diffp = sb.tile([128, 4, NBLK], f32, tag="diffp")
            for m in range(4):
                nc.vector.tensor_tensor(
                    diffp[:, m, :], rowp[:, m, :],
                    dexp4[:, m:m + 1].broadcast_to([128, NBLK]), A.subtract)
                nc.vector.tensor_reduce(zarg[:, m:m + 1], diffp[:, m, :],
                                        AX.X, A.add)
                nc.scalar.activation(logz[:, m:m + 1], zarg[:, m:m + 1], F.Ln)
            nc.vector.tensor_scalar_mul(
                logz[:, 4:5], pos_red[:], -SCALE / FP8_SCALE / 128.0)
            red1 = sb.tile([128, 1], f32, tag="red1")
            nc.vector.tensor_reduce(red1[:], logz[:], AX.X, A.add)
            fin = sb.tile([1, 1], f32, tag="fin")
            nc.gpsimd.tensor_reduce(fin[:], red1[:], AX.C, A.add)
            nc.sync.dma_start(out=out, in_=fin[:])

    from concourse import bacc as _bacc_mod

    orig_tables = _bacc_mod.get_activation_tables

    def _filtered(arch):
        tables = orig_tables(arch)
        keep = "natural_log_exp_and_others"
        F = mybir.ActivationFunctionType
        if (keep in tables and F.Exp in tables[keep]
                and F.Ln in tables[keep]):
            for name, fns in tables.items():
                if name != keep:
                    fns.discard(F.Exp)
                    fns.discard(F.Ln)
        return tables

    _bacc_mod.get_activation_tables = _filtered
    try:
        nc.compile()
    finally:
        _bacc_mod.get_activation_tables = orig_tables
    return nc


def _get_nc():
    if "nc" not in _CACHE:
        _CACHE["nc"] = _build()
    return _CACHE["nc"]


def _in_maps(z_i, z_j):
    import ml_dtypes

    z = np.concatenate(
        [np.asarray(z_i, np.float32), np.asarray(z_j, np.float32)], axis=0)
    zt = np.ascontiguousarray(z.T).astype(ml_dtypes.bfloat16)
    # [D, N2] -> [128(p), NBLK, KT, BLK]: per-partition contiguous
    ztH = np.ascontiguousarray(
        zt.reshape(KT, 128, NBLK, BLK).transpose(1, 2, 0, 3))

    def slc(off):
        s = zt[:, off:off + RPC]            # [D, RPC]
        return np.ascontiguousarray(s.reshape(KT, 128, RPC).transpose(1, 0, 2))

    maps = []
    for c in range(NCORES):
        o = c * RPC
        po = (o + B) % N2
        maps.append({
            "zt": ztH,
            "zown": slc(o),
            "zpr": slc(po),
        })
    return maps


def _run(z_i, z_j, trace=False):
    from concourse.bass_utils import run_bass_kernel_spmd

    nc = _get_nc()
    return run_bass_kernel_spmd(nc, _in_maps(z_i, z_j), list(range(NCORES)),
                                trace=trace)


def kernel(z_i, z_j):
    res = _run(z_i, z_j, trace=False)
    total = sum(float(r["out"][0, 0]) for r in res.results)
    return np.float32(total / N2)

